# revision 28
# baseline (speedup 1.0000x reference)
"""Pairwise cosine similarity  O = (Z/|Z_rows|) @ (Y/|Y_rows|).T  on 8 TRN2 cores.

Sharding: 4x2 output grid — core (zi, yi) holds a 1024-row Z shard and a
2048-row Y shard and computes the O^T block [2048, 1024]. Same FLOPs per
core as 1D sharding, ~half the bytes and transposes.

v18 structure (4x2 retry; the v14 corruption hit exactly the Y-tile XBAR
transposes queued behind the Z XBARs on the same queue):
  - Z XBAR transposes are ISOLATED on the sync queue; Y XBAR transposes
    run ONLY on the scalar queue (no Y-behind-Z adjacency anywhere).
  - the first 2 Y tiles (the observed corruption victims) are transposed
    on the PE instead (bf16 transpose -> bf16 PSUM -> DVE copyback),
    emitted after the previous chunk's matmuls so the PE never
    head-of-line blocks on loads.
  - inputs bf16 (host-converted); loads + outputs on gpsimd SWDGE
    (~16MB/iter). kxm is TWO [128, 32, 512] bf16 tiles (proven AP shape).
  - n_chunk=128 (1 y-subtile per chunk) so PSUM fits: 2 accs/gen x
    bufs=2 = 4 banks + 2 transpose-staging banks.
  - matmul: yt STATIONARY [128k, 128y]; kxmA/B MOVING [128k, 512z].
    1/|y| applied as per-partition activation scale at eviction; O^T
    blocks assembled and un-transposed on the host.
"""

import contextlib
import os
import sys
import numpy as np

_TRN_REPO = "/opt/trn_rl_repo"
if _TRN_REPO not in sys.path:
    sys.path.insert(0, _TRN_REPO)

import concourse.bacc as bacc
import concourse.mybir as mybir
import concourse.tile as tile
from concourse.bass_utils import run_bass_kernel_spmd
from concourse.masks import make_identity

P = 128
N_CORES = 8
F32 = mybir.dt.float32
BF16 = mybir.dt.bfloat16


def build(bz_core=1024, by=2048, feat=4096, n_chunk=128, bench_iters=None):
    """Build + bacc-compile the SPMD program (same program on every core)."""
    assert bz_core % 512 == 0 and by % n_chunk == 0 and feat % P == 0
    m_sub = bz_core // P          # z 128-row tiles
    k_tiles = feat // P           # contraction tiles
    n_chunks = by // n_chunk      # Y row chunks (1 tile each)
    h_sub = bz_core // 512        # 512-wide z slices (kxm tiles)
    PE_TR = {0, 1}                # chunks whose transpose runs on the PE

    nc = bacc.Bacc("TRN2", target_bir_lowering=False, debug=False,
                   num_devices=N_CORES)
    if bench_iters is None:
        z = nc.dram_tensor("z", [bz_core, feat], BF16, kind="ExternalInput").ap()
        y = nc.dram_tensor("y", [by, feat], BF16, kind="ExternalInput").ap()
        # o holds this core's O^T block [by, bz_core]
        o = nc.dram_tensor("o", [by, bz_core], BF16, kind="ExternalOutput").ap()
    else:
        # bench mode: no host I/O, garbage-content internal tensors
        z = nc.dram_tensor("zi", [bz_core, feat], BF16).ap()
        y = nc.dram_tensor("yi", [by, feat], BF16).ap()
        o = nc.dram_tensor("oi", [by, bz_core], BF16).ap()
        dummy_in = nc.dram_tensor("dummy_in", [1, 64], F32,
                                  kind="ExternalInput").ap()
        dummy_out = nc.dram_tensor("dummy_out", [1, 64], F32,
                                   kind="ExternalOutput").ap()

    with tile.TileContext(nc) as tc:
        with tc.tile_pool(name="const", bufs=1) as const_pool, \
             tc.tile_pool(name="kxm", bufs=1) as kxm_pool, \
             tc.tile_pool(name="zn", bufs=2) as zn_pool, \
             tc.tile_pool(name="nat", bufs=3) as nat_pool, \
             tc.tile_pool(name="small", bufs=2) as small_pool, \
             tc.tile_pool(name="sq", bufs=1) as sq_pool, \
             tc.tile_pool(name="yt", bufs=3) as yt_pool, \
             tc.tile_pool(name="outs", bufs=3) as out_pool, \
             tc.tile_pool(name="pacc", bufs=2, space="PSUM") as pacc_pool, \
             tc.tile_pool(name="ptr", bufs=2, space="PSUM") as ptr_pool:

            identf = const_pool.tile([P, P], F32, name="identf")
            make_identity(nc, identf)
            identb = const_pool.tile([P, P], BF16, name="identb")
            nc.vector.tensor_copy(identb[:], identf[:])

            if bench_iters is None:
                _loop = contextlib.nullcontext()
            else:
                _loop = tc.For_i(0, bench_iters, 1)
            with _loop:
                def row_rnorm(nat_ap, rdst, sq_out):
                    """rdst[p,0] = 1/|row p| for a [P, feat] bf16 tile."""
                    ss = small_pool.tile([P, 1], F32, tag="ss")
                    nc.scalar.activation(
                        sq_out, nat_ap,
                        mybir.ActivationFunctionType.Square,
                        accum_out=ss[:])
                    std = small_pool.tile([P, 1], F32, tag="std")
                    nc.scalar.sqrt(std[:], ss[:])
                    nc.vector.reciprocal(rdst, std[:])

                # ---- Z phase: norms + in-place prescale + XBAR into kxm ----
                # Z XBARs ride the sync queue EXCLUSIVELY.
                rz = small_pool.tile([P, m_sub], F32, tag="rz")
                # kxm[h][kk, k, zz] = Zn[h*512 + zz, k*128 + kk]
                kxms = [kxm_pool.tile([P, k_tiles, 512], BF16,
                                      name=f"kxm{h}", tag=f"kxm{h}")
                        for h in range(h_sub)]
                for j in range(m_sub):
                    zb = zn_pool.tile([P, feat], BF16, tag="zn")
                    nc.gpsimd.dma_start(out=zb[:],
                                        in_=z[j * P:(j + 1) * P, :])
                    zq = sq_pool.tile([P, feat], BF16, tag="sqt")
                    row_rnorm(zb[:], rz[:, j:j + 1], zq[:])
                    nc.vector.tensor_scalar_mul(zb[:], zb[:], rz[:, j:j + 1])
                    h, jj = divmod(j, 4)
                    nc.sync.dma_start_transpose(
                        kxms[h][:, :, jj * P:(jj + 1) * P], zb[:])

                # ---- main loop over Y chunks (lag-1 chunk pipeline) ----
                rys = {}
                accs = {}
                yts = {}
                ybfs = {}

                def start_chunk(c):
                    ry = small_pool.tile([P, 1], F32, tag="ry")
                    ybf = nat_pool.tile([P, feat], BF16, tag="nat")
                    # yt[kk, (k q)] = Yn[c*n_chunk + q, k*128 + kk]
                    yt = yt_pool.tile([P, k_tiles * P], BF16, tag="yt")
                    nc.gpsimd.dma_start(
                        out=ybf[:],
                        in_=y[c * n_chunk:(c + 1) * n_chunk, :])
                    if c not in PE_TR:
                        # Y XBARs ride the scalar queue EXCLUSIVELY
                        nc.scalar.dma_start_transpose(
                            yt[:].rearrange("p (k q) -> p k q", k=k_tiles),
                            ybf[:])
                        # in-place square destroys ybf after the XBAR read
                        row_rnorm(ybf[:], ry[:, 0:1], ybf[:])
                    else:
                        # PE transposes this chunk later; square to scratch
                        ysq = sq_pool.tile([P, feat], BF16, tag="sqt")
                        row_rnorm(ybf[:], ry[:, 0:1], ysq[:])
                        ybfs[c] = ybf
                    rys[c] = ry
                    yts[c] = yt
                    accs[c] = [pacc_pool.tile([P, 512], F32,
                                              tag=f"acc{h}", name=f"acc{h}")
                               for h in range(h_sub)]

                def emit_pe_transposes(c):
                    if c not in PE_TR:
                        return
                    yt = yts[c]
                    ybf = ybfs.pop(c)
                    for k0 in range(0, k_tiles, 8):
                        pt = ptr_pool.tile([P, 8 * P], BF16, tag="ptp")
                        for i in range(8):
                            nc.tensor.transpose(
                                pt[:, i * P:(i + 1) * P],
                                ybf[:, (k0 + i) * P:(k0 + i + 1) * P],
                                identb[:])
                        nc.vector.tensor_copy(
                            yt[:, k0 * P:(k0 + 8) * P], pt[:])

                def emit_matmuls(c):
                    yt = yts.pop(c)
                    for k in range(k_tiles):
                        for h in range(h_sub):
                            nc.tensor.matmul(
                                accs[c][h][:],
                                yt[:, k * P:(k + 1) * P],
                                kxms[h][:, k, :],
                                start=(k == 0),
                                stop=(k == k_tiles - 1))
                    evict_chunk(c)

                def evict_chunk(c):
                    ry = rys.pop(c)
                    ob = out_pool.tile([P, bz_core], BF16, tag="ob")
                    for h in range(h_sub):
                        nc.scalar.activation(
                            ob[:, h * 512:(h + 1) * 512],
                            accs[c][h][:],
                            mybir.ActivationFunctionType.Copy,
                            scale=ry[:, 0:1])
                    nc.gpsimd.dma_start(
                        out=o[c * n_chunk:(c + 1) * n_chunk, :],
                        in_=ob[:])
                    del accs[c]

                for c in range(n_chunks + 1):
                    if c < n_chunks:
                        start_chunk(c)
                    if c >= 1:
                        emit_matmuls(c - 1)
                    if c < n_chunks:
                        # after the previous chunk's matmuls so the PE never
                        # head-of-line blocks on this chunk's loads
                        emit_pe_transposes(c)

            if bench_iters is not None:
                db = small_pool.tile([1, 64], F32, tag="db", name="db")
                nc.gpsimd.dma_start(out=db[:], in_=dummy_in[:])
                nc.vector.tensor_copy(db[:], db[:])
                nc.gpsimd.dma_start(out=dummy_out[:], in_=db[:])

    nc.compile()
    return nc


_CACHE = {}


def _get_compiled():
    if "nc" not in _CACHE:
        _CACHE["nc"] = build()
    return _CACHE["nc"]


def kernel(Z, Y):
    from ml_dtypes import bfloat16
    Z = np.asarray(Z, dtype=np.float32).astype(bfloat16)
    Y = np.asarray(Y, dtype=np.float32).astype(bfloat16)
    bz, by_full = Z.shape[0], Y.shape[0]
    zs, ys = bz // 4, by_full // 2          # 4x2 grid shards
    nc = _get_compiled()
    in_maps = []
    for i in range(N_CORES):
        zi, yi = divmod(i, 2)
        in_maps.append(
            {"z": np.ascontiguousarray(Z[zi * zs:(zi + 1) * zs]),
             "y": np.ascontiguousarray(Y[yi * ys:(yi + 1) * ys])})
    res = run_bass_kernel_spmd(nc, in_maps, list(range(N_CORES)))
    # core (zi, yi) returns O^T block [ys, zs] in bf16; assemble + upcast
    out_t = np.empty((by_full, bz), dtype=np.float32)
    for i in range(N_CORES):
        zi, yi = divmod(i, 2)
        out_t[yi * ys:(yi + 1) * ys, zi * zs:(zi + 1) * zs] = \
            res.results[i]["o"].astype(np.float32)
    return np.ascontiguousarray(out_t.T)


# revision 29
# speedup vs baseline: 1.1491x; 1.1491x over previous
"""Pairwise cosine similarity  O = (Z/|Z_rows|) @ (Y/|Y_rows|).T  on 8 TRN2 cores.

Sharding: 4x2 output grid — core (zi, yi) holds a 1024-row Z shard and a
2048-row Y shard and computes the O^T block [2048, 1024]. Same FLOPs per
core as 1D sharding, ~half the bytes and transposes.

v18 structure (4x2 retry; the v14 corruption hit exactly the Y-tile XBAR
transposes queued behind the Z XBARs on the same queue):
  - Z XBAR transposes are ISOLATED on the sync queue; Y XBAR transposes
    run ONLY on the scalar queue (no Y-behind-Z adjacency anywhere).
  - the first 4 Y tiles (the observed corruption victims) are transposed
    on the PE instead (bf16 transpose -> bf16 PSUM -> DVE copyback),
    emitted after the previous chunk's matmuls so the PE never
    head-of-line blocks on loads.
  - inputs bf16 (host-converted); loads + outputs on gpsimd SWDGE
    (~16MB/iter). kxm is TWO [128, 32, 512] bf16 tiles (proven AP shape).
  - n_chunk=128 (1 y-subtile per chunk) so PSUM fits: 2 accs/gen x
    bufs=2 = 4 banks + 2 transpose-staging banks.
  - matmul: yt STATIONARY [128k, 128y]; kxmA/B MOVING [128k, 512z].
    1/|y| applied as per-partition activation scale at eviction; O^T
    blocks assembled and un-transposed on the host.
"""

import contextlib
import os
import sys
import numpy as np

_TRN_REPO = "/opt/trn_rl_repo"
if _TRN_REPO not in sys.path:
    sys.path.insert(0, _TRN_REPO)

import concourse.bacc as bacc
import concourse.mybir as mybir
import concourse.tile as tile
from concourse.bass_utils import run_bass_kernel_spmd
from concourse.masks import make_identity

P = 128
N_CORES = 8
F32 = mybir.dt.float32
BF16 = mybir.dt.bfloat16


def build(bz_core=1024, by=2048, feat=4096, n_chunk=128, bench_iters=None):
    """Build + bacc-compile the SPMD program (same program on every core)."""
    assert bz_core % 512 == 0 and by % n_chunk == 0 and feat % P == 0
    m_sub = bz_core // P          # z 128-row tiles
    k_tiles = feat // P           # contraction tiles
    n_chunks = by // n_chunk      # Y row chunks (1 tile each)
    h_sub = bz_core // 512        # 512-wide z slices (kxm tiles)
    PE_TR = {0, 1, 2, 3}          # chunks whose transpose runs on the PE

    nc = bacc.Bacc("TRN2", target_bir_lowering=False, debug=False,
                   num_devices=N_CORES)
    if bench_iters is None:
        z = nc.dram_tensor("z", [bz_core, feat], BF16, kind="ExternalInput").ap()
        y = nc.dram_tensor("y", [by, feat], BF16, kind="ExternalInput").ap()
        # o holds this core's O^T block [by, bz_core]
        o = nc.dram_tensor("o", [by, bz_core], BF16, kind="ExternalOutput").ap()
    else:
        # bench mode: no host I/O, garbage-content internal tensors
        z = nc.dram_tensor("zi", [bz_core, feat], BF16).ap()
        y = nc.dram_tensor("yi", [by, feat], BF16).ap()
        o = nc.dram_tensor("oi", [by, bz_core], BF16).ap()
        dummy_in = nc.dram_tensor("dummy_in", [1, 64], F32,
                                  kind="ExternalInput").ap()
        dummy_out = nc.dram_tensor("dummy_out", [1, 64], F32,
                                   kind="ExternalOutput").ap()

    with tile.TileContext(nc) as tc:
        with tc.tile_pool(name="const", bufs=1) as const_pool, \
             tc.tile_pool(name="kxm", bufs=1) as kxm_pool, \
             tc.tile_pool(name="zn", bufs=2) as zn_pool, \
             tc.tile_pool(name="nat", bufs=3) as nat_pool, \
             tc.tile_pool(name="small", bufs=2) as small_pool, \
             tc.tile_pool(name="sq", bufs=1) as sq_pool, \
             tc.tile_pool(name="yt", bufs=3) as yt_pool, \
             tc.tile_pool(name="outs", bufs=3) as out_pool, \
             tc.tile_pool(name="pacc", bufs=2, space="PSUM") as pacc_pool, \
             tc.tile_pool(name="ptr", bufs=2, space="PSUM") as ptr_pool:

            identf = const_pool.tile([P, P], F32, name="identf")
            make_identity(nc, identf)
            identb = const_pool.tile([P, P], BF16, name="identb")
            nc.vector.tensor_copy(identb[:], identf[:])

            if bench_iters is None:
                _loop = contextlib.nullcontext()
            else:
                _loop = tc.For_i(0, bench_iters, 1)
            with _loop:
                def row_rnorm(nat_ap, rdst, sq_out):
                    """rdst[p,0] = 1/|row p| for a [P, feat] bf16 tile."""
                    ss = small_pool.tile([P, 1], F32, tag="ss")
                    nc.scalar.activation(
                        sq_out, nat_ap,
                        mybir.ActivationFunctionType.Square,
                        accum_out=ss[:])
                    std = small_pool.tile([P, 1], F32, tag="std")
                    nc.scalar.sqrt(std[:], ss[:])
                    nc.vector.reciprocal(rdst, std[:])

                # ---- Z phase: norms + in-place prescale + XBAR into kxm ----
                # Z XBARs ride the sync queue EXCLUSIVELY.
                rz = small_pool.tile([P, m_sub], F32, tag="rz")
                # kxm[h][kk, k, zz] = Zn[h*512 + zz, k*128 + kk]
                kxms = [kxm_pool.tile([P, k_tiles, 512], BF16,
                                      name=f"kxm{h}", tag=f"kxm{h}")
                        for h in range(h_sub)]
                for j in range(m_sub):
                    zb = zn_pool.tile([P, feat], BF16, tag="zn")
                    nc.gpsimd.dma_start(out=zb[:],
                                        in_=z[j * P:(j + 1) * P, :])
                    zq = sq_pool.tile([P, feat], BF16, tag="sqt")
                    row_rnorm(zb[:], rz[:, j:j + 1], zq[:])
                    nc.vector.tensor_scalar_mul(zb[:], zb[:], rz[:, j:j + 1])
                    h, jj = divmod(j, 4)
                    nc.sync.dma_start_transpose(
                        kxms[h][:, :, jj * P:(jj + 1) * P], zb[:])

                # ---- main loop over Y chunks (lag-1 chunk pipeline) ----
                rys = {}
                accs = {}
                yts = {}
                ybfs = {}

                def start_chunk(c):
                    ry = small_pool.tile([P, 1], F32, tag="ry")
                    ybf = nat_pool.tile([P, feat], BF16, tag="nat")
                    # yt[kk, (k q)] = Yn[c*n_chunk + q, k*128 + kk]
                    yt = yt_pool.tile([P, k_tiles * P], BF16, tag="yt")
                    nc.gpsimd.dma_start(
                        out=ybf[:],
                        in_=y[c * n_chunk:(c + 1) * n_chunk, :])
                    if c not in PE_TR:
                        # Y XBARs ride the scalar queue EXCLUSIVELY
                        nc.scalar.dma_start_transpose(
                            yt[:].rearrange("p (k q) -> p k q", k=k_tiles),
                            ybf[:])
                        # in-place square destroys ybf after the XBAR read
                        row_rnorm(ybf[:], ry[:, 0:1], ybf[:])
                    else:
                        # PE transposes this chunk later; square to scratch
                        ysq = sq_pool.tile([P, feat], BF16, tag="sqt")
                        row_rnorm(ybf[:], ry[:, 0:1], ysq[:])
                        ybfs[c] = ybf
                    rys[c] = ry
                    yts[c] = yt
                    accs[c] = [pacc_pool.tile([P, 512], F32,
                                              tag=f"acc{h}", name=f"acc{h}")
                               for h in range(h_sub)]

                def emit_pe_transposes(c):
                    if c not in PE_TR:
                        return
                    yt = yts[c]
                    ybf = ybfs.pop(c)
                    for k0 in range(0, k_tiles, 8):
                        pt = ptr_pool.tile([P, 8 * P], BF16, tag="ptp")
                        for i in range(8):
                            nc.tensor.transpose(
                                pt[:, i * P:(i + 1) * P],
                                ybf[:, (k0 + i) * P:(k0 + i + 1) * P],
                                identb[:])
                        nc.vector.tensor_copy(
                            yt[:, k0 * P:(k0 + 8) * P], pt[:])

                def emit_matmuls(c):
                    yt = yts.pop(c)
                    for k in range(k_tiles):
                        for h in range(h_sub):
                            nc.tensor.matmul(
                                accs[c][h][:],
                                yt[:, k * P:(k + 1) * P],
                                kxms[h][:, k, :],
                                start=(k == 0),
                                stop=(k == k_tiles - 1))
                    evict_chunk(c)

                def evict_chunk(c):
                    ry = rys.pop(c)
                    ob = out_pool.tile([P, bz_core], BF16, tag="ob")
                    for h in range(h_sub):
                        nc.scalar.activation(
                            ob[:, h * 512:(h + 1) * 512],
                            accs[c][h][:],
                            mybir.ActivationFunctionType.Copy,
                            scale=ry[:, 0:1])
                    nc.gpsimd.dma_start(
                        out=o[c * n_chunk:(c + 1) * n_chunk, :],
                        in_=ob[:])
                    del accs[c]

                for c in range(n_chunks + 1):
                    if c < n_chunks:
                        start_chunk(c)
                    if c >= 1:
                        emit_matmuls(c - 1)
                    if c < n_chunks:
                        # after the previous chunk's matmuls so the PE never
                        # head-of-line blocks on this chunk's loads
                        emit_pe_transposes(c)

            if bench_iters is not None:
                db = small_pool.tile([1, 64], F32, tag="db", name="db")
                nc.gpsimd.dma_start(out=db[:], in_=dummy_in[:])
                nc.vector.tensor_copy(db[:], db[:])
                nc.gpsimd.dma_start(out=dummy_out[:], in_=db[:])

    nc.compile()
    return nc


_CACHE = {}


def _get_compiled():
    if "nc" not in _CACHE:
        _CACHE["nc"] = build()
    return _CACHE["nc"]


def kernel(Z, Y):
    from ml_dtypes import bfloat16
    Z = np.asarray(Z, dtype=np.float32).astype(bfloat16)
    Y = np.asarray(Y, dtype=np.float32).astype(bfloat16)
    bz, by_full = Z.shape[0], Y.shape[0]
    zs, ys = bz // 4, by_full // 2          # 4x2 grid shards
    nc = _get_compiled()
    in_maps = []
    for i in range(N_CORES):
        zi, yi = divmod(i, 2)
        in_maps.append(
            {"z": np.ascontiguousarray(Z[zi * zs:(zi + 1) * zs]),
             "y": np.ascontiguousarray(Y[yi * ys:(yi + 1) * ys])})
    res = run_bass_kernel_spmd(nc, in_maps, list(range(N_CORES)))
    # core (zi, yi) returns O^T block [ys, zs] in bf16; assemble + upcast
    out_t = np.empty((by_full, bz), dtype=np.float32)
    for i in range(N_CORES):
        zi, yi = divmod(i, 2)
        out_t[yi * ys:(yi + 1) * ys, zi * zs:(zi + 1) * zs] = \
            res.results[i]["o"].astype(np.float32)
    return np.ascontiguousarray(out_t.T)
